# revision 21
# baseline (speedup 1.0000x reference)
"""L2SquaredConv2d (1x1 conv) on 8 TRN2 NeuronCores.

out[b,p,h,w] = relu( sum_c x[b,c,h,w]^2 - 2*sum_c x[b,c,h,w]*w[p,c] + sum_c w[p,c]^2 )

Strategy: data-parallel over batch (B=32 -> 4 images/core). Per core one big
matmul [P=2000, C=512] x [C, N=3136] in fp8(e4m3) with perf_mode=DoubleRow
(2 fp8 weights/PE cell -> 256-deep contraction per pass, ~2x bf16 FLOP rate).
The -2 factor is pre-folded into the weights on the host, w2[p] = sum_c w^2
is computed on the host (f32).

i2[n] = sum_c x^2 comes from a DoubleRow matmul of host-precomputed fp8 x^2
against an all-ones stationary (reduction + partition-broadcast in one pass),
evicted once to bf16. A burst of tiny ones*ones matmuls right after the
preamble warms the PE HAM clock gate (1.2 -> 2.4 GHz) before real work lands.

PSUM is organized as [128, 2, 1024] image-pair tiles (4 banks, 2 bufs), so
most p-chunks evict with 2 big ACT instructions (amortizing ~200ns/instr
overhead). Eviction is split across engines to balance:
  14 p-chunks: ScalarE u = Identity(psum + w2[p]) -> bf16; VectorE z = u+i2r;
               relu on GpSimd (8) / VectorE (6)
   2 p-chunks: VectorE scalar_tensor_tensor z = (psum + w2[p]) + i2r straight
               from PSUM; relu on GpSimd / per-image VectorE (short tail for
               the last chunk, which also stores per-image).
"""

import numpy as np
import ml_dtypes

import concourse.bacc as bacc
import concourse.bass as bass
import concourse.mybir as mybir
import concourse.tile as tile
from concourse import bass_utils

B, C, H, W = 32, 512, 28, 28
P = 2000
NCORES = 8
BL = B // NCORES          # 4 images per core
HW = H * W                # 784
N = BL * HW               # 3136 pixels per core
KC = C // 128             # 4 contraction chunks (2 DoubleRow pairs)
PC = (P + 127) // 128     # 16 p-chunks (last one is 80 rows)
P_PAD = PC * 128
NWARM = 30                # HAM warm-up matmuls

BF16 = mybir.dt.bfloat16
F32 = mybir.dt.float32
FP8 = mybir.dt.float8e4
NPBF16 = ml_dtypes.bfloat16
NPFP8 = ml_dtypes.float8_e4m3

DVE_CHUNKS = (7, 15)      # p-chunks evicted via VectorE stt from PSUM

_CACHE = {}


def _build():
    nc = bacc.Bacc(
        "TRN2", target_bir_lowering=False, debug=False, num_devices=NCORES
    )
    # partition-major layouts: [128, KC, cols]
    xT_d = nc.dram_tensor("xT", [128, KC, N], FP8, kind="ExternalInput")
    x2T_d = nc.dram_tensor("x2T", [128, KC, N], FP8, kind="ExternalInput")
    wT_d = nc.dram_tensor("wT", [128, KC, P_PAD], FP8, kind="ExternalInput")
    w2c_d = nc.dram_tensor("w2c", [128, PC], F32, kind="ExternalInput")
    out_d = nc.dram_tensor("out", [P, BL, HW], BF16, kind="ExternalOutput")

    IDENT = mybir.ActivationFunctionType.Identity
    COPY = mybir.ActivationFunctionType.Copy
    DR = mybir.MatmulPerfMode.DoubleRow
    ADD = mybir.AluOpType.add

    with tile.TileContext(nc) as tc:
        with (
            tc.tile_pool(name="resident", bufs=1) as rpool,
            tc.tile_pool(name="u", bufs=3) as u_pool,
            tc.tile_pool(name="z", bufs=2) as z_pool,
            tc.tile_pool(name="o", bufs=3) as o_pool,
            tc.tile_pool(name="pm", bufs=2, space=bass.MemorySpace.PSUM) as pm_pool,
        ):
            # ---- resident tiles ----
            x_sb = rpool.tile([128, KC, N], FP8, tag="x")
            x2_sb = rpool.tile([128, KC, N], FP8, tag="x2")
            wt_sb = rpool.tile([128, KC, P_PAD], FP8, tag="wt")
            ones_sb = rpool.tile([128, 2, 128], FP8, tag="ones")
            w2col = rpool.tile([128, PC], F32, tag="w2col")
            i2r = rpool.tile([128, BL, HW], BF16, tag="i2r")
            wscr = rpool.tile([128, 128], BF16, tag="wscr")

            # ones via memset (no DMA dependency -> warm-up can start at once)
            nc.gpsimd.memset(ones_sb[:], 1.0)

            # ---- PE warm-up burst: drives HAM to K=8/8 before real work ----
            wps = pm_pool.tile([128, 2, 1024], F32, tag="ps", name="warm")
            for i in range(NWARM):
                nc.tensor.matmul(
                    wps[:, 0, 0:128], ones_sb[:], ones_sb[:],
                    start=(i == 0), stop=(i == NWARM - 1),
                    perf_mode=DR,
                )
            nc.scalar.activation(wscr[:], wps[:, 0, 0:128], COPY)

            # ---- input DMAs, ordered so compute can start early ----
            nc.sync.dma_start(x2_sb[:, :, 0:HW], x2T_d[:, :, 0:HW])
            nc.sync.dma_start(wt_sb[:, :, 0:512], wT_d[:, :, 0:512])
            nc.sync.dma_start(x_sb[:, :, 0:HW], xT_d[:, :, 0:HW])
            nc.sync.dma_start(x_sb[:, :, HW:N], xT_d[:, :, HW:N])
            nc.sync.dma_start(x2_sb[:, :, HW:N], x2T_d[:, :, HW:N])
            nc.sync.dma_start(wt_sb[:, :, 512:P_PAD], wT_d[:, :, 512:P_PAD])
            nc.sync.dma_start(w2col[:], w2c_d[:])

            def i2_pair(t):
                """i2 rows for images (2t, 2t+1) via ones.T @ x2 DoubleRow."""
                pi = pm_pool.tile([128, 2, 1024], F32, tag="ps", name="pi")
                for kk in range(2):
                    for j in range(2):
                        base = (2 * t + j) * HW
                        for off, nn in ((0, 512), (512, 272)):
                            nc.tensor.matmul(
                                pi[:, j, off:off + nn],
                                ones_sb[:],
                                x2_sb[:, 2 * kk:2 * kk + 2,
                                      base + off:base + off + nn],
                                start=(kk == 0), stop=(kk == 1),
                                perf_mode=DR,
                            )
                for j in range(2):
                    nc.scalar.activation(
                        i2r[:, 2 * t + j, :], pi[:, j, 0:HW], COPY
                    )

            def main_chunk(p_i, defer_late_imgs=False):
                M = min(128, P - p_i * 128)
                psl = slice(p_i * 128, p_i * 128 + M)
                ps = [pm_pool.tile([128, 2, 1024], F32, tag="ps", name=f"ps{t}")
                      for t in range(2)]
                z = z_pool.tile([128, BL, HW], BF16)
                u = (u_pool.tile([128, BL, HW], BF16, name="u")
                     if p_i not in DVE_CHUNKS else None)
                # pair-major so pair 0's PSUM completes early and its
                # eviction overlaps pair 1's matmuls
                for t in range(2):
                    for kk in range(2):
                        for j in range(2):
                            base = (2 * t + j) * HW
                            for off, nn in ((0, 512), (512, 272)):
                                nc.tensor.matmul(
                                    ps[t][:M, j, off:off + nn],
                                    wt_sb[:, 2 * kk:2 * kk + 2, psl],
                                    x_sb[:, 2 * kk:2 * kk + 2,
                                         base + off:base + off + nn],
                                    start=(kk == 0), stop=(kk == 1),
                                    perf_mode=DR,
                                )
                    if p_i not in DVE_CHUNKS:
                        # ScalarE eviction (+w2 bias); VectorE adds i2 after
                        for j in range(2):
                            nc.scalar.activation(
                                u[:M, 2 * t + j, :], ps[t][:M, j, 0:HW],
                                IDENT, bias=w2col[:M, p_i:p_i + 1], scale=1.0,
                            )
                    else:
                        # VectorE stt eviction straight from PSUM: (ps+w2)+i2r
                        for j in range(2):
                            img = 2 * t + j
                            nc.vector.scalar_tensor_tensor(
                                z[:M, img, :], ps[t][:M, j, 0:HW],
                                w2col[:M, p_i:p_i + 1], i2r[:M, img, :],
                                op0=ADD, op1=ADD,
                            )
                def finish():
                    if p_i not in DVE_CHUNKS:
                        if defer_late_imgs:
                            # images 0-1 now, 2-3 once i2_pair(1) has run
                            nc.vector.tensor_add(
                                z[:M, 0:2, :], u[:M, 0:2, :], i2r[:M, 0:2, :]
                            )
                            return lambda: _tail_after_add(True)
                        nc.vector.tensor_add(z[:M], u[:M], i2r[:M])
                    return _tail_after_add(False)

                def _tail_after_add(late_half):
                    if late_half:
                        nc.vector.tensor_add(
                            z[:M, 2:4, :], u[:M, 2:4, :], i2r[:M, 2:4, :]
                        )
                    o = o_pool.tile([128, BL, HW], BF16, name="o")
                    if p_i == PC - 1:
                        # short tail: per-image relu + store
                        for img in range(BL):
                            nc.vector.tensor_scalar_max(
                                o[:M, img, :], z[:M, img, :], 0.0
                            )
                            nc.sync.dma_start(
                                out_d[psl, img:img + 1, :],
                                o[:M, img:img + 1, :]
                            )
                    else:
                        nc.vector.tensor_scalar_max(o[:M], z[:M], 0.0)
                        nc.sync.dma_start(out_d[psl], o[:M])

                return finish()

            # ---- schedule: i2 for images 0-1 first (x2 cols 0:HW are the
            # first DMA; images 2-3 need the late x2 upper half), then the
            # main p-chunks; p0/p1 defer their images-2-3 i2 add until
            # i2_pair(1) has actually written those rows.
            i2_pair(0)
            fin0 = main_chunk(0, defer_late_imgs=True)
            fin1 = main_chunk(1, defer_late_imgs=True)
            i2_pair(1)
            fin0()
            fin1()
            for p_i in range(2, PC):
                main_chunk(p_i)

    nc.compile()
    return nc


def _get_nc():
    if "nc" not in _CACHE:
        _CACHE["nc"] = _build()
    return _CACHE["nc"]


def _make_in_maps(input, weights):
    x = np.asarray(input, dtype=np.float32)
    w = np.asarray(weights, dtype=np.float32).reshape(P, C)

    wm2 = (-2.0 * w).astype(NPFP8)                      # [P, C] fp8 of -2w
    wT = np.zeros((C, P_PAD), NPFP8)
    wT[:, :P] = wm2.T
    # [C, P_PAD] -> [KC, 128, P_PAD] -> partition-major [128, KC, P_PAD]
    wT = np.ascontiguousarray(wT.reshape(KC, 128, P_PAD).transpose(1, 0, 2))

    w2 = np.einsum("pc,pc->p", w.astype(np.float64), w.astype(np.float64))
    w2c = np.zeros(P_PAD, np.float32)
    w2c[:P] = w2.astype(np.float32)
    w2c = np.ascontiguousarray(w2c.reshape(PC, 128).T)  # [128, PC]

    in_maps = []
    for c in range(NCORES):
        sh = x[c * BL:(c + 1) * BL]                     # [4, 512, 28, 28]
        xt32 = np.ascontiguousarray(
            sh.transpose(1, 0, 2, 3).reshape(C, N)
        )
        xT = np.ascontiguousarray(
            xt32.astype(NPFP8).reshape(KC, 128, N).transpose(1, 0, 2)
        )
        x2T = np.ascontiguousarray(
            (xt32 * xt32).astype(NPFP8).reshape(KC, 128, N).transpose(1, 0, 2)
        )
        in_maps.append({"xT": xT, "x2T": x2T, "wT": wT, "w2c": w2c})
    return in_maps


def run(input, weights, trace=False):
    """Returns (output [32,2000,28,28] f32, BassKernelResults)."""
    nc = _get_nc()
    in_maps = _make_in_maps(input, weights)
    res = bass_utils.run_bass_kernel_spmd(
        nc, in_maps, core_ids=list(range(NCORES)), trace=trace
    )
    outs = [res.results[c]["out"] for c in range(NCORES)]   # [2000, 4, 784] bf16
    out = (
        np.stack(outs, axis=0)                              # [8, 2000, 4, 784]
        .transpose(0, 2, 1, 3)                              # [8, 4, 2000, 784]
        .astype(np.float32)
        .reshape(B, P, H, W)
    )
    return out, res


def kernel(input, weights):
    out, _ = run(input, weights, trace=False)
    return out


# revision 24
# speedup vs baseline: 1.1053x; 1.1053x over previous
"""L2SquaredConv2d (1x1 conv) on 8 TRN2 NeuronCores.

out[b,p,h,w] = relu( sum_c x[b,c,h,w]^2 - 2*sum_c x[b,c,h,w]*w[p,c] + sum_c w[p,c]^2 )

Strategy: data-parallel over batch (B=32 -> 4 images/core). Per core one big
matmul [P=2000, C=512] x [C, N=3136] in fp8(e4m3) with perf_mode=DoubleRow
(2 fp8 weights/PE cell -> 256-deep contraction per pass, ~2x bf16 FLOP rate).
The -2 factor is pre-folded into the weights on the host, w2[p] = sum_c w^2
is computed on the host (f32).

i2[n] = sum_c x^2 comes from a DoubleRow matmul of host-precomputed fp8 x^2
against an all-ones stationary (reduction + partition-broadcast in one pass),
evicted once to bf16. A burst of tiny ones*ones matmuls right after the
preamble warms the PE HAM clock gate (1.2 -> 2.4 GHz) before real work lands.

PSUM is organized as [128, 2, 1024] image-pair tiles (4 banks, 2 bufs), so
most p-chunks evict with 2 big ACT instructions (amortizing ~200ns/instr
overhead). Eviction is split across engines to balance:
  14 p-chunks: ScalarE u = Identity(psum + w2[p]) -> bf16; VectorE z = u+i2r;
               relu on GpSimd (8) / VectorE (6)
   2 p-chunks: VectorE scalar_tensor_tensor z = (psum + w2[p]) + i2r straight
               from PSUM; relu on GpSimd / per-image VectorE (short tail for
               the last chunk, which also stores per-image).
"""

import numpy as np
import ml_dtypes

import concourse.bacc as bacc
import concourse.bass as bass
import concourse.mybir as mybir
import concourse.tile as tile
from concourse import bass_utils

B, C, H, W = 32, 512, 28, 28
P = 2000
NCORES = 8
BL = B // NCORES          # 4 images per core
HW = H * W                # 784
N = BL * HW               # 3136 pixels per core
KC = C // 128             # 4 contraction chunks (2 DoubleRow pairs)
PC = (P + 127) // 128     # 16 p-chunks (last one is 80 rows)
P_PAD = PC * 128
NWARM = 30                # HAM warm-up matmuls

BF16 = mybir.dt.bfloat16
F32 = mybir.dt.float32
FP8 = mybir.dt.float8e4
NPBF16 = ml_dtypes.bfloat16
NPFP8 = ml_dtypes.float8_e4m3

DVE_CHUNKS = (4, 9, 15)   # p-chunks evicted via VectorE stt from PSUM

_CACHE = {}


def _build():
    nc = bacc.Bacc(
        "TRN2", target_bir_lowering=False, debug=False, num_devices=NCORES
    )
    # partition-major layouts: [128, KC, cols]
    xT_d = nc.dram_tensor("xT", [128, KC, N], FP8, kind="ExternalInput")
    x2T_d = nc.dram_tensor("x2T", [128, KC, N], FP8, kind="ExternalInput")
    wT_d = nc.dram_tensor("wT", [128, KC, P_PAD], FP8, kind="ExternalInput")
    w2c_d = nc.dram_tensor("w2c", [128, PC], F32, kind="ExternalInput")
    out_d = nc.dram_tensor("out", [P, BL, HW], BF16, kind="ExternalOutput")

    IDENT = mybir.ActivationFunctionType.Identity
    COPY = mybir.ActivationFunctionType.Copy
    DR = mybir.MatmulPerfMode.DoubleRow
    ADD = mybir.AluOpType.add

    with tile.TileContext(nc) as tc:
        with (
            tc.tile_pool(name="resident", bufs=1) as rpool,
            tc.tile_pool(name="u", bufs=3) as u_pool,
            tc.tile_pool(name="z", bufs=2) as z_pool,
            tc.tile_pool(name="o", bufs=3) as o_pool,
            tc.tile_pool(name="pm", bufs=2, space=bass.MemorySpace.PSUM) as pm_pool,
        ):
            # ---- resident tiles ----
            x_sb = rpool.tile([128, KC, N], FP8, tag="x")
            x2_sb = rpool.tile([128, KC, N], FP8, tag="x2")
            wt_sb = rpool.tile([128, KC, P_PAD], FP8, tag="wt")
            ones_sb = rpool.tile([128, 2, 128], FP8, tag="ones")
            w2col = rpool.tile([128, PC], F32, tag="w2col")
            i2r = rpool.tile([128, BL, HW], BF16, tag="i2r")
            wscr = rpool.tile([128, 128], BF16, tag="wscr")

            # ones via memset (no DMA dependency -> warm-up can start at once)
            nc.gpsimd.memset(ones_sb[:], 1.0)

            # ---- PE warm-up burst: drives HAM to K=8/8 before real work ----
            wps = pm_pool.tile([128, 2, 1024], F32, tag="ps", name="warm")
            for i in range(NWARM):
                nc.tensor.matmul(
                    wps[:, 0, 0:128], ones_sb[:], ones_sb[:],
                    start=(i == 0), stop=(i == NWARM - 1),
                    perf_mode=DR,
                )
            nc.scalar.activation(wscr[:], wps[:, 0, 0:128], COPY)

            # ---- input DMAs, ordered so compute can start early ----
            nc.sync.dma_start(x2_sb[:, :, 0:HW], x2T_d[:, :, 0:HW])
            nc.sync.dma_start(wt_sb[:, :, 0:512], wT_d[:, :, 0:512])
            nc.sync.dma_start(x_sb[:, :, 0:HW], xT_d[:, :, 0:HW])
            nc.sync.dma_start(x_sb[:, :, HW:N], xT_d[:, :, HW:N])
            nc.sync.dma_start(x2_sb[:, :, HW:N], x2T_d[:, :, HW:N])
            nc.sync.dma_start(wt_sb[:, :, 512:P_PAD], wT_d[:, :, 512:P_PAD])
            nc.sync.dma_start(w2col[:], w2c_d[:])

            def i2_pair(t):
                """i2 rows for images (2t, 2t+1) via ones.T @ x2 DoubleRow."""
                pi = pm_pool.tile([128, 2, 1024], F32, tag="ps", name="pi")
                for kk in range(2):
                    for j in range(2):
                        base = (2 * t + j) * HW
                        for off, nn in ((0, 512), (512, 272)):
                            nc.tensor.matmul(
                                pi[:, j, off:off + nn],
                                ones_sb[:],
                                x2_sb[:, 2 * kk:2 * kk + 2,
                                      base + off:base + off + nn],
                                start=(kk == 0), stop=(kk == 1),
                                perf_mode=DR,
                            )
                nc.scalar.activation(
                    i2r[:, 2 * t:2 * t + 2, :], pi[:, :, 0:HW], COPY
                )

            def main_chunk(p_i, defer_late_imgs=False):
                M = min(128, P - p_i * 128)
                psl = slice(p_i * 128, p_i * 128 + M)
                ps = [pm_pool.tile([128, 2, 1024], F32, tag="ps", name=f"ps{t}")
                      for t in range(2)]
                z = z_pool.tile([128, BL, HW], BF16)
                u = (u_pool.tile([128, BL, HW], BF16, name="u")
                     if p_i not in DVE_CHUNKS else None)
                # image-major so each image's PSUM completes after 4 matmuls
                # and its eviction overlaps the remaining matmuls tightly
                for t in range(2):
                    for j in range(2):
                        base = (2 * t + j) * HW
                        for off, nn in ((0, 512), (512, 272)):
                            for kk in range(2):
                                nc.tensor.matmul(
                                    ps[t][:M, j, off:off + nn],
                                    wt_sb[:, 2 * kk:2 * kk + 2, psl],
                                    x_sb[:, 2 * kk:2 * kk + 2,
                                         base + off:base + off + nn],
                                    start=(kk == 0), stop=(kk == 1),
                                    perf_mode=DR,
                                )
                        if p_i in DVE_CHUNKS:
                            # VectorE stt straight from PSUM: (ps+w2)+i2r
                            img = 2 * t + j
                            nc.vector.scalar_tensor_tensor(
                                z[:M, img, :], ps[t][:M, j, 0:HW],
                                w2col[:M, p_i:p_i + 1], i2r[:M, img, :],
                                op0=ADD, op1=ADD,
                            )
                    if p_i not in DVE_CHUNKS:
                        # ScalarE pair eviction (+w2 bias); VectorE adds i2
                        nc.scalar.activation(
                            u[:M, 2 * t:2 * t + 2, :], ps[t][:M, :, 0:HW],
                            IDENT, bias=w2col[:M, p_i:p_i + 1], scale=1.0,
                        )
                def finish():
                    if p_i not in DVE_CHUNKS:
                        if defer_late_imgs:
                            # images 0-1 now, 2-3 once i2_pair(1) has run
                            nc.vector.tensor_add(
                                z[:M, 0:2, :], u[:M, 0:2, :], i2r[:M, 0:2, :]
                            )
                            return lambda: _tail_after_add(True)
                        nc.vector.tensor_add(z[:M], u[:M], i2r[:M])
                    return _tail_after_add(False)

                def _tail_after_add(late_half):
                    if late_half:
                        nc.vector.tensor_add(
                            z[:M, 2:4, :], u[:M, 2:4, :], i2r[:M, 2:4, :]
                        )
                    o = o_pool.tile([128, BL, HW], BF16, name="o")
                    if p_i == PC - 1:
                        # short tail: per-image relu + store
                        for img in range(BL):
                            nc.vector.tensor_scalar_max(
                                o[:M, img, :], z[:M, img, :], 0.0
                            )
                            nc.sync.dma_start(
                                out_d[psl, img:img + 1, :],
                                o[:M, img:img + 1, :]
                            )
                    else:
                        nc.vector.tensor_scalar_max(o[:M], z[:M], 0.0)
                        nc.sync.dma_start(out_d[psl], o[:M])

                return finish()

            # ---- schedule: i2 for images 0-1 first (x2 cols 0:HW are the
            # first DMA; images 2-3 need the late x2 upper half), then the
            # main p-chunks; p0/p1 defer their images-2-3 i2 add until
            # i2_pair(1) has actually written those rows.
            i2_pair(0)
            fin0 = main_chunk(0, defer_late_imgs=True)
            fin1 = main_chunk(1, defer_late_imgs=True)
            i2_pair(1)
            fin0()
            fin1()
            for p_i in range(2, PC):
                main_chunk(p_i)

    nc.compile()
    return nc


def _get_nc():
    if "nc" not in _CACHE:
        _CACHE["nc"] = _build()
    return _CACHE["nc"]


def _make_in_maps(input, weights):
    x = np.asarray(input, dtype=np.float32)
    w = np.asarray(weights, dtype=np.float32).reshape(P, C)

    wm2 = (-2.0 * w).astype(NPFP8)                      # [P, C] fp8 of -2w
    wT = np.zeros((C, P_PAD), NPFP8)
    wT[:, :P] = wm2.T
    # [C, P_PAD] -> [KC, 128, P_PAD] -> partition-major [128, KC, P_PAD]
    wT = np.ascontiguousarray(wT.reshape(KC, 128, P_PAD).transpose(1, 0, 2))

    w2 = np.einsum("pc,pc->p", w.astype(np.float64), w.astype(np.float64))
    w2c = np.zeros(P_PAD, np.float32)
    w2c[:P] = w2.astype(np.float32)
    w2c = np.ascontiguousarray(w2c.reshape(PC, 128).T)  # [128, PC]

    in_maps = []
    for c in range(NCORES):
        sh = x[c * BL:(c + 1) * BL]                     # [4, 512, 28, 28]
        xt32 = np.ascontiguousarray(
            sh.transpose(1, 0, 2, 3).reshape(C, N)
        )
        xT = np.ascontiguousarray(
            xt32.astype(NPFP8).reshape(KC, 128, N).transpose(1, 0, 2)
        )
        x2T = np.ascontiguousarray(
            (xt32 * xt32).astype(NPFP8).reshape(KC, 128, N).transpose(1, 0, 2)
        )
        in_maps.append({"xT": xT, "x2T": x2T, "wT": wT, "w2c": w2c})
    return in_maps


def run(input, weights, trace=False):
    """Returns (output [32,2000,28,28] f32, BassKernelResults)."""
    nc = _get_nc()
    in_maps = _make_in_maps(input, weights)
    res = bass_utils.run_bass_kernel_spmd(
        nc, in_maps, core_ids=list(range(NCORES)), trace=trace
    )
    outs = [res.results[c]["out"] for c in range(NCORES)]   # [2000, 4, 784] bf16
    out = (
        np.stack(outs, axis=0)                              # [8, 2000, 4, 784]
        .transpose(0, 2, 1, 3)                              # [8, 4, 2000, 784]
        .astype(np.float32)
        .reshape(B, P, H, W)
    )
    return out, res


def kernel(input, weights):
    out, _ = run(input, weights, trace=False)
    return out


# revision 29
# speedup vs baseline: 1.1995x; 1.0852x over previous
"""L2SquaredConv2d (1x1 conv) on 8 TRN2 NeuronCores.

out[b,p,h,w] = relu( sum_c x[b,c,h,w]^2 - 2*sum_c x[b,c,h,w]*w[p,c] + sum_c w[p,c]^2 )

Strategy: data-parallel over batch (B=32 -> 4 images/core). Per core one big
matmul [P=2000, C=512] x [C, N=3136] in fp8(e4m3) with perf_mode=DoubleRow
(2 fp8 weights/PE cell -> 256-deep contraction per pass, ~2x bf16 FLOP rate).
The -2 factor is pre-folded into the weights on the host, w2[p] = sum_c w^2
is computed on the host (f32).

i2[n] = sum_c x^2 comes from a DoubleRow matmul of host-precomputed fp8 x^2
against an all-ones stationary (reduction + partition-broadcast in one pass),
evicted once to bf16. A burst of tiny ones*ones matmuls right after the
preamble warms the PE HAM clock gate (1.2 -> 2.4 GHz) before real work lands.

PSUM is organized as [128, 2, 1024] image-pair tiles (4 banks, 2 bufs), so
most p-chunks evict with 2 big ACT instructions (amortizing ~200ns/instr
overhead). Eviction is split across engines to balance:
  14 p-chunks: ScalarE u = Identity(psum + w2[p]) -> bf16; VectorE z = u+i2r;
               relu on GpSimd (8) / VectorE (6)
   2 p-chunks: VectorE scalar_tensor_tensor z = (psum + w2[p]) + i2r straight
               from PSUM; relu on GpSimd / per-image VectorE (short tail for
               the last chunk, which also stores per-image).
"""

import numpy as np
import ml_dtypes

import concourse.bacc as bacc
import concourse.bass as bass
import concourse.mybir as mybir
import concourse.tile as tile
from concourse import bass_utils

B, C, H, W = 32, 512, 28, 28
P = 2000
NCORES = 8
BL = B // NCORES          # 4 images per core
HW = H * W                # 784
N = BL * HW               # 3136 pixels per core
KC = C // 128             # 4 contraction chunks (2 DoubleRow pairs)
PC = (P + 127) // 128     # 16 p-chunks (last one is 80 rows)
P_PAD = PC * 128
NWARM = 56                # HAM warm-up matmuls (~7us: spans 2 HAM windows)

BF16 = mybir.dt.bfloat16
F32 = mybir.dt.float32
FP8 = mybir.dt.float8e4
NPBF16 = ml_dtypes.bfloat16
NPFP8 = ml_dtypes.float8_e4m3

DVE_CHUNKS = (4, 9, 15)   # p-chunks evicted via VectorE stt from PSUM

_CACHE = {}


def _build():
    nc = bacc.Bacc(
        "TRN2", target_bir_lowering=False, debug=False, num_devices=NCORES
    )
    # partition-major layouts: [128, KC, cols]
    xT_d = nc.dram_tensor("xT", [128, KC, N], FP8, kind="ExternalInput")
    x2T_d = nc.dram_tensor("x2T", [128, KC, N], FP8, kind="ExternalInput")
    wT_d = nc.dram_tensor("wT", [128, KC, P_PAD], FP8, kind="ExternalInput")
    w2c_d = nc.dram_tensor("w2c", [128, PC], F32, kind="ExternalInput")
    out_d = nc.dram_tensor("out", [P, BL, HW], BF16, kind="ExternalOutput")

    IDENT = mybir.ActivationFunctionType.Identity
    COPY = mybir.ActivationFunctionType.Copy
    DR = mybir.MatmulPerfMode.DoubleRow
    ADD = mybir.AluOpType.add

    with tile.TileContext(nc) as tc:
        with (
            tc.tile_pool(name="resident", bufs=1) as rpool,
            tc.tile_pool(name="u", bufs=3) as u_pool,
            tc.tile_pool(name="z", bufs=2) as z_pool,
            tc.tile_pool(name="o", bufs=3) as o_pool,
            tc.tile_pool(name="pm", bufs=2, space=bass.MemorySpace.PSUM) as pm_pool,
        ):
            # ---- resident tiles ----
            x_sb = rpool.tile([128, KC, N], FP8, tag="x")
            x2_sb = rpool.tile([128, KC, N], FP8, tag="x2")
            wt_sb = rpool.tile([128, KC, P_PAD], FP8, tag="wt")
            ones_sb = rpool.tile([128, 2, 128], FP8, tag="ones")
            w2col = rpool.tile([128, PC], F32, tag="w2col")
            i2r = rpool.tile([128, BL, HW], BF16, tag="i2r")
            wscr = rpool.tile([128, 128], BF16, tag="wscr")

            # ones via memset (no DMA dependency -> warm-up can start at once)
            nc.gpsimd.memset(ones_sb[:], 1.0)

            # ---- PE warm-up burst: drives HAM to K=8/8 before real work ----
            wps = pm_pool.tile([128, 2, 1024], F32, tag="ps", name="warm")
            for i in range(NWARM):
                nc.tensor.matmul(
                    wps[:, 0, 0:128], ones_sb[:], ones_sb[:],
                    start=(i == 0), stop=(i == NWARM - 1),
                    perf_mode=DR,
                )
            nc.scalar.activation(wscr[:], wps[:, 0, 0:128], COPY)

            # ---- input DMAs, ordered so compute can start early ----
            nc.sync.dma_start(x2_sb[:, :, 0:HW], x2T_d[:, :, 0:HW])
            nc.sync.dma_start(wt_sb[:, :, 0:512], wT_d[:, :, 0:512])
            nc.sync.dma_start(x_sb[:, :, 0:HW], xT_d[:, :, 0:HW])
            nc.sync.dma_start(x_sb[:, :, HW:N], xT_d[:, :, HW:N])
            nc.sync.dma_start(x2_sb[:, :, HW:N], x2T_d[:, :, HW:N])
            nc.sync.dma_start(wt_sb[:, :, 512:P_PAD], wT_d[:, :, 512:P_PAD])
            nc.sync.dma_start(w2col[:], w2c_d[:])

            def i2_pair(t):
                """i2 rows for images (2t, 2t+1) via ones.T @ x2 DoubleRow."""
                pi = pm_pool.tile([128, 2, 1024], F32, tag="ps", name="pi")
                for kk in range(2):
                    for j in range(2):
                        base = (2 * t + j) * HW
                        for off, nn in ((0, 512), (512, 272)):
                            nc.tensor.matmul(
                                pi[:, j, off:off + nn],
                                ones_sb[:],
                                x2_sb[:, 2 * kk:2 * kk + 2,
                                      base + off:base + off + nn],
                                start=(kk == 0), stop=(kk == 1),
                                perf_mode=DR,
                            )
                nc.scalar.activation(
                    i2r[:, 2 * t:2 * t + 2, :], pi[:, :, 0:HW], COPY
                )

            def main_chunk(p_i):
                M = min(128, P - p_i * 128)
                psl = slice(p_i * 128, p_i * 128 + M)
                ps = [pm_pool.tile([128, 2, 1024], F32, tag="ps", name=f"ps{t}")
                      for t in range(2)]
                z = z_pool.tile([128, BL, HW], BF16)
                u = (u_pool.tile([128, BL, HW], BF16, name="u")
                     if p_i not in DVE_CHUNKS else None)
                # image-major so each image's PSUM completes after 4 matmuls
                # and its eviction overlaps the remaining matmuls tightly
                for t in range(2):
                    for j in range(2):
                        base = (2 * t + j) * HW
                        for off, nn in ((0, 512), (512, 272)):
                            for kk in range(2):
                                nc.tensor.matmul(
                                    ps[t][:M, j, off:off + nn],
                                    wt_sb[:, 2 * kk:2 * kk + 2, psl],
                                    x_sb[:, 2 * kk:2 * kk + 2,
                                         base + off:base + off + nn],
                                    start=(kk == 0), stop=(kk == 1),
                                    perf_mode=DR,
                                )
                        if p_i in DVE_CHUNKS:
                            # VectorE stt straight from PSUM: (ps+w2)+i2r
                            img = 2 * t + j
                            nc.vector.scalar_tensor_tensor(
                                z[:M, img, :], ps[t][:M, j, 0:HW],
                                w2col[:M, p_i:p_i + 1], i2r[:M, img, :],
                                op0=ADD, op1=ADD,
                            )
                    if p_i not in DVE_CHUNKS:
                        # ScalarE pair eviction (+w2 bias); VectorE adds i2
                        nc.scalar.activation(
                            u[:M, 2 * t:2 * t + 2, :], ps[t][:M, :, 0:HW],
                            IDENT, bias=w2col[:M, p_i:p_i + 1], scale=1.0,
                        )
                def finish():
                    """z-add (a-chunks) + relu + store. Emitted one chunk
                    late so PSUM-critical DVE work stays at the queue head."""
                    if p_i not in DVE_CHUNKS:
                        nc.vector.tensor_add(z[:M], u[:M], i2r[:M])
                    o = o_pool.tile([128, BL, HW], BF16, name="o")
                    if p_i == PC - 1:
                        # short tail: per-image relu + store
                        for img in range(BL):
                            nc.vector.tensor_scalar_max(
                                o[:M, img, :], z[:M, img, :], 0.0
                            )
                            nc.sync.dma_start(
                                out_d[psl, img:img + 1, :],
                                o[:M, img:img + 1, :]
                            )
                    else:
                        nc.vector.tensor_scalar_max(o[:M], z[:M], 0.0)
                        nc.sync.dma_start(out_d[psl], o[:M])

                return finish

            # ---- schedule: i2 pair 0 first (x2 cols 0:HW are the first
            # DMA; images 2-3 need the late x2 upper half, so i2 pair 1 runs
            # after chunk 1's matmuls). Each chunk's z/relu/store chain is
            # emitted after the NEXT chunk's matmuls+evictions, so the
            # PSUM-freeing work is never queued behind bulk DVE work.
            i2_pair(0)
            fins = [main_chunk(0), main_chunk(1)]
            i2_pair(1)
            fins.pop(0)()                      # finish(0)
            for p_i in range(2, PC):
                fins.append(main_chunk(p_i))
                fins.pop(0)()                  # finish(p_i - 1)
            fins.pop(0)()                      # finish(PC - 1)

    nc.compile()
    return nc


def _get_nc():
    if "nc" not in _CACHE:
        _CACHE["nc"] = _build()
    return _CACHE["nc"]


def _make_in_maps(input, weights):
    x = np.asarray(input, dtype=np.float32)
    w = np.asarray(weights, dtype=np.float32).reshape(P, C)

    wm2 = (-2.0 * w).astype(NPFP8)                      # [P, C] fp8 of -2w
    wT = np.zeros((C, P_PAD), NPFP8)
    wT[:, :P] = wm2.T
    # [C, P_PAD] -> [KC, 128, P_PAD] -> partition-major [128, KC, P_PAD]
    wT = np.ascontiguousarray(wT.reshape(KC, 128, P_PAD).transpose(1, 0, 2))

    w2 = np.einsum("pc,pc->p", w.astype(np.float64), w.astype(np.float64))
    w2c = np.zeros(P_PAD, np.float32)
    w2c[:P] = w2.astype(np.float32)
    w2c = np.ascontiguousarray(w2c.reshape(PC, 128).T)  # [128, PC]

    in_maps = []
    for c in range(NCORES):
        sh = x[c * BL:(c + 1) * BL]                     # [4, 512, 28, 28]
        xt32 = np.ascontiguousarray(
            sh.transpose(1, 0, 2, 3).reshape(C, N)
        )
        xT = np.ascontiguousarray(
            xt32.astype(NPFP8).reshape(KC, 128, N).transpose(1, 0, 2)
        )
        x2T = np.ascontiguousarray(
            (xt32 * xt32).astype(NPFP8).reshape(KC, 128, N).transpose(1, 0, 2)
        )
        in_maps.append({"xT": xT, "x2T": x2T, "wT": wT, "w2c": w2c})
    return in_maps


def run(input, weights, trace=False):
    """Returns (output [32,2000,28,28] f32, BassKernelResults)."""
    nc = _get_nc()
    in_maps = _make_in_maps(input, weights)
    res = bass_utils.run_bass_kernel_spmd(
        nc, in_maps, core_ids=list(range(NCORES)), trace=trace
    )
    outs = [res.results[c]["out"] for c in range(NCORES)]   # [2000, 4, 784] bf16
    out = (
        np.stack(outs, axis=0)                              # [8, 2000, 4, 784]
        .transpose(0, 2, 1, 3)                              # [8, 4, 2000, 784]
        .astype(np.float32)
        .reshape(B, P, H, W)
    )
    return out, res


def kernel(input, weights):
    out, _ = run(input, weights, trace=False)
    return out


# revision 34
# speedup vs baseline: 1.2248x; 1.0211x over previous
"""L2SquaredConv2d (1x1 conv) on 8 TRN2 NeuronCores.

out[b,p,h,w] = relu( sum_c x[b,c,h,w]^2 - 2*sum_c x[b,c,h,w]*w[p,c] + sum_c w[p,c]^2 )

Strategy: data-parallel over batch (B=32 -> 4 images/core). Per core one big
matmul [P=2000, C=512] x [C, N=3136] in fp8(e4m3) with perf_mode=DoubleRow
(2 fp8 weights/PE cell -> 256-deep contraction per pass, ~2x bf16 FLOP rate).
The -2 factor is pre-folded into the weights on the host, w2[p] = sum_c w^2
is computed on the host (f32).

i2[n] = sum_c x^2 comes from a DoubleRow matmul of host-precomputed fp8 x^2
against an all-ones stationary (reduction + partition-broadcast in one pass),
evicted once to bf16. A burst of tiny ones*ones matmuls right after the
preamble warms the PE HAM clock gate (1.2 -> 2.4 GHz) before real work lands.

PSUM is organized as [128, 2, 1024] image-pair tiles (4 banks, 2 bufs), so
most p-chunks evict with 2 big ACT instructions (amortizing ~200ns/instr
overhead). Eviction is split across engines to balance:
  14 p-chunks: ScalarE u = Identity(psum + w2[p]) -> bf16; VectorE z = u+i2r;
               relu on GpSimd (8) / VectorE (6)
   2 p-chunks: VectorE scalar_tensor_tensor z = (psum + w2[p]) + i2r straight
               from PSUM; relu on GpSimd / per-image VectorE (short tail for
               the last chunk, which also stores per-image).
"""

import numpy as np
import ml_dtypes

import concourse.bacc as bacc
import concourse.bass as bass
import concourse.mybir as mybir
import concourse.tile as tile
from concourse import bass_utils

B, C, H, W = 32, 512, 28, 28
P = 2000
NCORES = 8
BL = B // NCORES          # 4 images per core
HW = H * W                # 784
N = BL * HW               # 3136 pixels per core
KC = C // 128             # 4 contraction chunks (2 DoubleRow pairs)
PC = (P + 127) // 128     # 16 p-chunks (last one is 80 rows)
P_PAD = PC * 128
NWARM = 56                # HAM warm-up matmuls (~7us: spans 2 HAM windows)

BF16 = mybir.dt.bfloat16
F32 = mybir.dt.float32
FP8 = mybir.dt.float8e4
NPBF16 = ml_dtypes.bfloat16
NPFP8 = ml_dtypes.float8_e4m3

DVE_CHUNKS = (4, 9)       # p-chunks evicted via VectorE stt from PSUM

_CACHE = {}


def _build():
    nc = bacc.Bacc(
        "TRN2", target_bir_lowering=False, debug=False, num_devices=NCORES
    )
    # partition-major layouts: [128, KC, cols]
    xT_d = nc.dram_tensor("xT", [128, KC, N], FP8, kind="ExternalInput")
    x2T_d = nc.dram_tensor("x2T", [128, KC, N], FP8, kind="ExternalInput")
    wT_d = nc.dram_tensor("wT", [128, KC, P_PAD], FP8, kind="ExternalInput")
    w2c_d = nc.dram_tensor("w2c", [128, PC], F32, kind="ExternalInput")
    out_d = nc.dram_tensor("out", [P, BL, HW], BF16, kind="ExternalOutput")

    IDENT = mybir.ActivationFunctionType.Identity
    COPY = mybir.ActivationFunctionType.Copy
    DR = mybir.MatmulPerfMode.DoubleRow
    ADD = mybir.AluOpType.add

    with tile.TileContext(nc) as tc:
        with (
            tc.tile_pool(name="resident", bufs=1) as rpool,
            tc.tile_pool(name="u", bufs=3) as u_pool,
            tc.tile_pool(name="z", bufs=2) as z_pool,
            tc.tile_pool(name="o", bufs=3) as o_pool,
            tc.tile_pool(name="pm", bufs=2, space=bass.MemorySpace.PSUM) as pm_pool,
        ):
            # ---- resident tiles ----
            x_sb = rpool.tile([128, KC, N], FP8, tag="x")
            x2_sb = rpool.tile([128, KC, N], FP8, tag="x2")
            wt_sb = rpool.tile([128, KC, P_PAD], FP8, tag="wt")
            ones_sb = rpool.tile([128, 2, 128], FP8, tag="ones")
            w2col = rpool.tile([128, PC], F32, tag="w2col")
            i2r = rpool.tile([128, BL, HW], BF16, tag="i2r")
            wscr = rpool.tile([128, 128], BF16, tag="wscr")

            # ones via memset (no DMA dependency -> warm-up can start at once)
            nc.gpsimd.memset(ones_sb[:], 1.0)

            # ---- PE warm-up burst: drives HAM to K=8/8 before real work ----
            wps = pm_pool.tile([128, 2, 1024], F32, tag="ps", name="warm")
            for i in range(NWARM):
                nc.tensor.matmul(
                    wps[:, 0, 0:128], ones_sb[:], ones_sb[:],
                    start=(i == 0), stop=(i == NWARM - 1),
                    perf_mode=DR,
                )
            nc.scalar.activation(wscr[:], wps[:, 0, 0:128], COPY)

            # ---- input DMAs, ordered so compute can start early ----
            nc.sync.dma_start(x2_sb[:, :, 0:2 * HW], x2T_d[:, :, 0:2 * HW])
            nc.sync.dma_start(wt_sb[:, :, 0:512], wT_d[:, :, 0:512])
            for img in range(BL):
                nc.sync.dma_start(
                    x_sb[:, :, img * HW:(img + 1) * HW],
                    xT_d[:, :, img * HW:(img + 1) * HW],
                )
            nc.sync.dma_start(x2_sb[:, :, 2 * HW:N], x2T_d[:, :, 2 * HW:N])
            nc.sync.dma_start(wt_sb[:, :, 512:P_PAD], wT_d[:, :, 512:P_PAD])
            nc.sync.dma_start(w2col[:], w2c_d[:])

            def i2_pair(t):
                """i2 rows for images (2t, 2t+1) via ones.T @ x2 DoubleRow."""
                pi = pm_pool.tile([128, 2, 1024], F32, tag="ps", name="pi")
                for kk in range(2):
                    for j in range(2):
                        base = (2 * t + j) * HW
                        for off, nn in ((0, 512), (512, 272)):
                            nc.tensor.matmul(
                                pi[:, j, off:off + nn],
                                ones_sb[:],
                                x2_sb[:, 2 * kk:2 * kk + 2,
                                      base + off:base + off + nn],
                                start=(kk == 0), stop=(kk == 1),
                                perf_mode=DR,
                            )
                nc.scalar.activation(
                    i2r[:, 2 * t:2 * t + 2, :], pi[:, :, 0:HW], COPY
                )

            def main_chunk(p_i):
                M = min(128, P - p_i * 128)
                psl = slice(p_i * 128, p_i * 128 + M)
                ps = [pm_pool.tile([128, 2, 1024], F32, tag="ps", name=f"ps{t}")
                      for t in range(2)]
                z = z_pool.tile([128, BL, HW], BF16)
                u = (u_pool.tile([128, BL, HW], BF16, name="u")
                     if p_i not in DVE_CHUNKS else None)
                # image-major so each image's PSUM completes after 4 matmuls
                # and its eviction overlaps the remaining matmuls tightly
                for t in range(2):
                    for j in range(2):
                        base = (2 * t + j) * HW
                        for off, nn in ((0, 512), (512, 272)):
                            for kk in range(2):
                                nc.tensor.matmul(
                                    ps[t][:M, j, off:off + nn],
                                    wt_sb[:, 2 * kk:2 * kk + 2, psl],
                                    x_sb[:, 2 * kk:2 * kk + 2,
                                         base + off:base + off + nn],
                                    start=(kk == 0), stop=(kk == 1),
                                    perf_mode=DR,
                                )
                        if p_i in DVE_CHUNKS:
                            # VectorE stt straight from PSUM: (ps+w2)+i2r
                            img = 2 * t + j
                            nc.vector.scalar_tensor_tensor(
                                z[:M, img, :], ps[t][:M, j, 0:HW],
                                w2col[:M, p_i:p_i + 1], i2r[:M, img, :],
                                op0=ADD, op1=ADD,
                            )
                    if p_i not in DVE_CHUNKS:
                        # ScalarE pair eviction (+w2 bias); VectorE adds i2
                        nc.scalar.activation(
                            u[:M, 2 * t:2 * t + 2, :], ps[t][:M, :, 0:HW],
                            IDENT, bias=w2col[:M, p_i:p_i + 1], scale=1.0,
                        )
                def finish():
                    """z-add (a-chunks) + relu + store. Emitted one chunk
                    late so PSUM-critical DVE work stays at the queue head."""
                    if p_i not in DVE_CHUNKS and p_i != PC - 1:
                        nc.vector.tensor_add(z[:M], u[:M], i2r[:M])
                    o = o_pool.tile([128, BL, HW], BF16, name="o")
                    if p_i == PC - 1:
                        # short tail: per-image i2-add + relu + store
                        for img in range(BL):
                            nc.vector.tensor_add(
                                z[:M, img, :], u[:M, img, :], i2r[:M, img, :]
                            )
                            nc.vector.tensor_scalar_max(
                                o[:M, img, :], z[:M, img, :], 0.0
                            )
                            nc.sync.dma_start(
                                out_d[psl, img:img + 1, :],
                                o[:M, img:img + 1, :]
                            )
                    else:
                        nc.vector.tensor_scalar_max(o[:M], z[:M], 0.0)
                        nc.sync.dma_start(out_d[psl], o[:M])

                return finish

            # ---- schedule: i2 pair 0 first (x2 cols 0:HW are the first
            # DMA; images 2-3 need the late x2 upper half, so i2 pair 1 runs
            # after chunk 1's matmuls). Each chunk's z/relu/store chain is
            # emitted after the NEXT chunk's matmuls+evictions, so the
            # PSUM-freeing work is never queued behind bulk DVE work.
            i2_pair(0)
            fins = [main_chunk(0), main_chunk(1)]
            i2_pair(1)
            fins.pop(0)()                      # finish(0)
            for p_i in range(2, PC):
                fins.append(main_chunk(p_i))
                if p_i < PC - 1:
                    fins.pop(0)()              # finish(p_i - 1)
            fins.pop(1)()                      # finish(15): short tail first
            fins.pop(0)()                      # finish(14)

    nc.compile()
    return nc


def _get_nc():
    if "nc" not in _CACHE:
        _CACHE["nc"] = _build()
    return _CACHE["nc"]


def _make_in_maps(input, weights):
    x = np.asarray(input, dtype=np.float32)
    w = np.asarray(weights, dtype=np.float32).reshape(P, C)

    wm2 = (-2.0 * w).astype(NPFP8)                      # [P, C] fp8 of -2w
    wT = np.zeros((C, P_PAD), NPFP8)
    wT[:, :P] = wm2.T
    # [C, P_PAD] -> [KC, 128, P_PAD] -> partition-major [128, KC, P_PAD]
    wT = np.ascontiguousarray(wT.reshape(KC, 128, P_PAD).transpose(1, 0, 2))

    w2 = np.einsum("pc,pc->p", w.astype(np.float64), w.astype(np.float64))
    w2c = np.zeros(P_PAD, np.float32)
    w2c[:P] = w2.astype(np.float32)
    w2c = np.ascontiguousarray(w2c.reshape(PC, 128).T)  # [128, PC]

    in_maps = []
    for c in range(NCORES):
        sh = x[c * BL:(c + 1) * BL]                     # [4, 512, 28, 28]
        xt32 = np.ascontiguousarray(
            sh.transpose(1, 0, 2, 3).reshape(C, N)
        )
        xT = np.ascontiguousarray(
            xt32.astype(NPFP8).reshape(KC, 128, N).transpose(1, 0, 2)
        )
        x2T = np.ascontiguousarray(
            (xt32 * xt32).astype(NPFP8).reshape(KC, 128, N).transpose(1, 0, 2)
        )
        in_maps.append({"xT": xT, "x2T": x2T, "wT": wT, "w2c": w2c})
    return in_maps


def run(input, weights, trace=False):
    """Returns (output [32,2000,28,28] f32, BassKernelResults)."""
    nc = _get_nc()
    in_maps = _make_in_maps(input, weights)
    res = bass_utils.run_bass_kernel_spmd(
        nc, in_maps, core_ids=list(range(NCORES)), trace=trace
    )
    outs = [res.results[c]["out"] for c in range(NCORES)]   # [2000, 4, 784] bf16
    out = (
        np.stack(outs, axis=0)                              # [8, 2000, 4, 784]
        .transpose(0, 2, 1, 3)                              # [8, 4, 2000, 784]
        .astype(np.float32)
        .reshape(B, P, H, W)
    )
    return out, res


def kernel(input, weights):
    out, _ = run(input, weights, trace=False)
    return out


# revision 35
# speedup vs baseline: 1.2280x; 1.0026x over previous
"""L2SquaredConv2d (1x1 conv) on 8 TRN2 NeuronCores.

out[b,p,h,w] = relu( sum_c x[b,c,h,w]^2 - 2*sum_c x[b,c,h,w]*w[p,c] + sum_c w[p,c]^2 )

Strategy: data-parallel over batch (B=32 -> 4 images/core). Per core one big
matmul [P=2000, C=512] x [C, N=3136] in fp8(e4m3) with perf_mode=DoubleRow
(2 fp8 weights/PE cell -> 256-deep contraction per pass, ~2x bf16 FLOP rate).
The -2 factor is pre-folded into the weights on the host, w2[p] = sum_c w^2
is computed on the host (f32).

i2[n] = sum_c x^2 comes from a DoubleRow matmul of host-precomputed fp8 x^2
against an all-ones stationary (reduction + partition-broadcast in one pass),
evicted once to bf16. A burst of tiny ones*ones matmuls right after the
preamble warms the PE HAM clock gate (1.2 -> 2.4 GHz) before real work lands.

PSUM is organized as [128, 2, 1024] image-pair tiles (4 banks, 2 bufs), so
most p-chunks evict with 2 big ACT instructions (amortizing ~200ns/instr
overhead). Eviction is split across engines to balance:
  14 p-chunks: ScalarE u = Identity(psum + w2[p]) -> bf16; VectorE z = u+i2r;
               relu on GpSimd (8) / VectorE (6)
   2 p-chunks: VectorE scalar_tensor_tensor z = (psum + w2[p]) + i2r straight
               from PSUM; relu on GpSimd / per-image VectorE (short tail for
               the last chunk, which also stores per-image).
"""

import numpy as np
import ml_dtypes

import concourse.bacc as bacc
import concourse.bass as bass
import concourse.mybir as mybir
import concourse.tile as tile
from concourse import bass_utils

B, C, H, W = 32, 512, 28, 28
P = 2000
NCORES = 8
BL = B // NCORES          # 4 images per core
HW = H * W                # 784
N = BL * HW               # 3136 pixels per core
KC = C // 128             # 4 contraction chunks (2 DoubleRow pairs)
PC = (P + 127) // 128     # 16 p-chunks (last one is 80 rows)
P_PAD = PC * 128
NWARM = 85                # HAM warm-up matmuls: bridges the input-DMA phase

BF16 = mybir.dt.bfloat16
F32 = mybir.dt.float32
FP8 = mybir.dt.float8e4
NPBF16 = ml_dtypes.bfloat16
NPFP8 = ml_dtypes.float8_e4m3

DVE_CHUNKS = (4, 9)       # p-chunks evicted via VectorE stt from PSUM

_CACHE = {}


def _build():
    nc = bacc.Bacc(
        "TRN2", target_bir_lowering=False, debug=False, num_devices=NCORES
    )
    # partition-major layouts: [128, KC, cols]
    xT_d = nc.dram_tensor("xT", [128, KC, N], FP8, kind="ExternalInput")
    x2T_d = nc.dram_tensor("x2T", [128, KC, N], FP8, kind="ExternalInput")
    wT_d = nc.dram_tensor("wT", [128, KC, P_PAD], FP8, kind="ExternalInput")
    w2c_d = nc.dram_tensor("w2c", [128, PC], F32, kind="ExternalInput")
    out_d = nc.dram_tensor("out", [P, BL, HW], BF16, kind="ExternalOutput")

    IDENT = mybir.ActivationFunctionType.Identity
    COPY = mybir.ActivationFunctionType.Copy
    DR = mybir.MatmulPerfMode.DoubleRow
    ADD = mybir.AluOpType.add

    with tile.TileContext(nc) as tc:
        with (
            tc.tile_pool(name="resident", bufs=1) as rpool,
            tc.tile_pool(name="u", bufs=3) as u_pool,
            tc.tile_pool(name="z", bufs=2) as z_pool,
            tc.tile_pool(name="o", bufs=3) as o_pool,
            tc.tile_pool(name="pm", bufs=2, space=bass.MemorySpace.PSUM) as pm_pool,
        ):
            # ---- resident tiles ----
            x_sb = rpool.tile([128, KC, N], FP8, tag="x")
            x2_sb = rpool.tile([128, KC, N], FP8, tag="x2")
            wt_sb = rpool.tile([128, KC, P_PAD], FP8, tag="wt")
            ones_sb = rpool.tile([128, 2, 128], FP8, tag="ones")
            w2col = rpool.tile([128, PC], F32, tag="w2col")
            i2r = rpool.tile([128, BL, HW], BF16, tag="i2r")
            wscr = rpool.tile([128, 128], BF16, tag="wscr")

            # ones via memset (no DMA dependency -> warm-up can start at once)
            nc.gpsimd.memset(ones_sb[:], 1.0)

            # ---- PE warm-up burst: drives HAM to K=8/8 before real work ----
            wps = pm_pool.tile([128, 2, 1024], F32, tag="ps", name="warm")
            for i in range(NWARM):
                nc.tensor.matmul(
                    wps[:, 0, 0:128], ones_sb[:], ones_sb[:],
                    start=(i == 0), stop=(i == NWARM - 1),
                    perf_mode=DR,
                )
            nc.scalar.activation(wscr[:], wps[:, 0, 0:128], COPY)

            # ---- input DMAs, ordered so compute can start early ----
            nc.sync.dma_start(x2_sb[:, :, 0:2 * HW], x2T_d[:, :, 0:2 * HW])
            nc.sync.dma_start(wt_sb[:, :, 0:512], wT_d[:, :, 0:512])
            for img in range(BL):
                nc.sync.dma_start(
                    x_sb[:, :, img * HW:(img + 1) * HW],
                    xT_d[:, :, img * HW:(img + 1) * HW],
                )
            nc.sync.dma_start(x2_sb[:, :, 2 * HW:N], x2T_d[:, :, 2 * HW:N])
            nc.sync.dma_start(wt_sb[:, :, 512:P_PAD], wT_d[:, :, 512:P_PAD])
            nc.sync.dma_start(w2col[:], w2c_d[:])

            def i2_pair(t):
                """i2 rows for images (2t, 2t+1) via ones.T @ x2 DoubleRow."""
                pi = pm_pool.tile([128, 2, 1024], F32, tag="ps", name="pi")
                for kk in range(2):
                    for j in range(2):
                        base = (2 * t + j) * HW
                        for off, nn in ((0, 512), (512, 272)):
                            nc.tensor.matmul(
                                pi[:, j, off:off + nn],
                                ones_sb[:],
                                x2_sb[:, 2 * kk:2 * kk + 2,
                                      base + off:base + off + nn],
                                start=(kk == 0), stop=(kk == 1),
                                perf_mode=DR,
                            )
                nc.scalar.activation(
                    i2r[:, 2 * t:2 * t + 2, :], pi[:, :, 0:HW], COPY
                )

            def main_chunk(p_i):
                M = min(128, P - p_i * 128)
                psl = slice(p_i * 128, p_i * 128 + M)
                ps = [pm_pool.tile([128, 2, 1024], F32, tag="ps", name=f"ps{t}")
                      for t in range(2)]
                z = z_pool.tile([128, BL, HW], BF16)
                u = (u_pool.tile([128, BL, HW], BF16, name="u")
                     if p_i not in DVE_CHUNKS else None)
                # image-major so each image's PSUM completes after 4 matmuls
                # and its eviction overlaps the remaining matmuls tightly
                for t in range(2):
                    for j in range(2):
                        base = (2 * t + j) * HW
                        for off, nn in ((0, 512), (512, 272)):
                            for kk in range(2):
                                nc.tensor.matmul(
                                    ps[t][:M, j, off:off + nn],
                                    wt_sb[:, 2 * kk:2 * kk + 2, psl],
                                    x_sb[:, 2 * kk:2 * kk + 2,
                                         base + off:base + off + nn],
                                    start=(kk == 0), stop=(kk == 1),
                                    perf_mode=DR,
                                )
                        if p_i in DVE_CHUNKS:
                            # VectorE stt straight from PSUM: (ps+w2)+i2r
                            img = 2 * t + j
                            nc.vector.scalar_tensor_tensor(
                                z[:M, img, :], ps[t][:M, j, 0:HW],
                                w2col[:M, p_i:p_i + 1], i2r[:M, img, :],
                                op0=ADD, op1=ADD,
                            )
                    if p_i not in DVE_CHUNKS:
                        # ScalarE pair eviction (+w2 bias); VectorE adds i2
                        nc.scalar.activation(
                            u[:M, 2 * t:2 * t + 2, :], ps[t][:M, :, 0:HW],
                            IDENT, bias=w2col[:M, p_i:p_i + 1], scale=1.0,
                        )
                def finish():
                    """z-add (a-chunks) + relu + store. Emitted one chunk
                    late so PSUM-critical DVE work stays at the queue head."""
                    if p_i not in DVE_CHUNKS and p_i != PC - 1:
                        nc.vector.tensor_add(z[:M], u[:M], i2r[:M])
                    o = o_pool.tile([128, BL, HW], BF16, name="o")
                    if p_i == PC - 1:
                        # short tail: per-image i2-add + relu + store
                        for img in range(BL):
                            nc.vector.tensor_add(
                                z[:M, img, :], u[:M, img, :], i2r[:M, img, :]
                            )
                            nc.vector.tensor_scalar_max(
                                o[:M, img, :], z[:M, img, :], 0.0
                            )
                            nc.sync.dma_start(
                                out_d[psl, img:img + 1, :],
                                o[:M, img:img + 1, :]
                            )
                    else:
                        nc.vector.tensor_scalar_max(o[:M], z[:M], 0.0)
                        nc.sync.dma_start(out_d[psl], o[:M])

                return finish

            # ---- schedule: i2 pair 0 first (x2 cols 0:HW are the first
            # DMA; images 2-3 need the late x2 upper half, so i2 pair 1 runs
            # after chunk 1's matmuls). Each chunk's z/relu/store chain is
            # emitted after the NEXT chunk's matmuls+evictions, so the
            # PSUM-freeing work is never queued behind bulk DVE work.
            i2_pair(0)
            fins = [main_chunk(0), main_chunk(1)]
            i2_pair(1)
            fins.pop(0)()                      # finish(0)
            for p_i in range(2, PC):
                fins.append(main_chunk(p_i))
                if p_i < PC - 1:
                    fins.pop(0)()              # finish(p_i - 1)
            fins.pop(1)()                      # finish(15): short tail first
            fins.pop(0)()                      # finish(14)

    nc.compile()
    return nc


def _get_nc():
    if "nc" not in _CACHE:
        _CACHE["nc"] = _build()
    return _CACHE["nc"]


def _make_in_maps(input, weights):
    x = np.asarray(input, dtype=np.float32)
    w = np.asarray(weights, dtype=np.float32).reshape(P, C)

    wm2 = (-2.0 * w).astype(NPFP8)                      # [P, C] fp8 of -2w
    wT = np.zeros((C, P_PAD), NPFP8)
    wT[:, :P] = wm2.T
    # [C, P_PAD] -> [KC, 128, P_PAD] -> partition-major [128, KC, P_PAD]
    wT = np.ascontiguousarray(wT.reshape(KC, 128, P_PAD).transpose(1, 0, 2))

    w2 = np.einsum("pc,pc->p", w.astype(np.float64), w.astype(np.float64))
    w2c = np.zeros(P_PAD, np.float32)
    w2c[:P] = w2.astype(np.float32)
    w2c = np.ascontiguousarray(w2c.reshape(PC, 128).T)  # [128, PC]

    in_maps = []
    for c in range(NCORES):
        sh = x[c * BL:(c + 1) * BL]                     # [4, 512, 28, 28]
        xt32 = np.ascontiguousarray(
            sh.transpose(1, 0, 2, 3).reshape(C, N)
        )
        xT = np.ascontiguousarray(
            xt32.astype(NPFP8).reshape(KC, 128, N).transpose(1, 0, 2)
        )
        x2T = np.ascontiguousarray(
            (xt32 * xt32).astype(NPFP8).reshape(KC, 128, N).transpose(1, 0, 2)
        )
        in_maps.append({"xT": xT, "x2T": x2T, "wT": wT, "w2c": w2c})
    return in_maps


def run(input, weights, trace=False):
    """Returns (output [32,2000,28,28] f32, BassKernelResults)."""
    nc = _get_nc()
    in_maps = _make_in_maps(input, weights)
    res = bass_utils.run_bass_kernel_spmd(
        nc, in_maps, core_ids=list(range(NCORES)), trace=trace
    )
    outs = [res.results[c]["out"] for c in range(NCORES)]   # [2000, 4, 784] bf16
    out = (
        np.stack(outs, axis=0)                              # [8, 2000, 4, 784]
        .transpose(0, 2, 1, 3)                              # [8, 4, 2000, 784]
        .astype(np.float32)
        .reshape(B, P, H, W)
    )
    return out, res


def kernel(input, weights):
    out, _ = run(input, weights, trace=False)
    return out


# revision 37
# speedup vs baseline: 1.2297x; 1.0013x over previous
"""L2SquaredConv2d (1x1 conv) on 8 TRN2 NeuronCores.

out[b,p,h,w] = relu( sum_c x[b,c,h,w]^2 - 2*sum_c x[b,c,h,w]*w[p,c] + sum_c w[p,c]^2 )

Strategy: data-parallel over batch (B=32 -> 4 images/core). Per core one big
matmul [P=2000, C=512] x [C, N=3136] in fp8(e4m3) with perf_mode=DoubleRow
(2 fp8 weights/PE cell -> 256-deep contraction per pass, ~2x bf16 FLOP rate).
The -2 factor is pre-folded into the weights on the host, w2[p] = sum_c w^2
is computed on the host (f32).

i2[n] = sum_c x^2 comes from a DoubleRow matmul of host-precomputed fp8 x^2
against an all-ones stationary (reduction + partition-broadcast in one pass),
evicted once to bf16. A burst of tiny ones*ones matmuls right after the
preamble warms the PE HAM clock gate (1.2 -> 2.4 GHz) before real work lands.

PSUM is organized as [128, 2, 1024] image-pair tiles (4 banks, 2 bufs), so
most p-chunks evict with 2 big ACT instructions (amortizing ~200ns/instr
overhead). Eviction is split across engines to balance:
  14 p-chunks: ScalarE u = Identity(psum + w2[p]) -> bf16; VectorE z = u+i2r;
               relu on GpSimd (8) / VectorE (6)
   2 p-chunks: VectorE scalar_tensor_tensor z = (psum + w2[p]) + i2r straight
               from PSUM; relu on GpSimd / per-image VectorE (short tail for
               the last chunk, which also stores per-image).
"""

import numpy as np
import ml_dtypes

import concourse.bacc as bacc
import concourse.bass as bass
import concourse.mybir as mybir
import concourse.tile as tile
from concourse import bass_utils

B, C, H, W = 32, 512, 28, 28
P = 2000
NCORES = 8
BL = B // NCORES          # 4 images per core
HW = H * W                # 784
N = BL * HW               # 3136 pixels per core
KC = C // 128             # 4 contraction chunks (2 DoubleRow pairs)
PC = (P + 127) // 128     # 16 p-chunks (last one is 80 rows)
P_PAD = PC * 128
NWARM = 62                # HAM warm-up matmuls: bridges the input-DMA phase

BF16 = mybir.dt.bfloat16
F32 = mybir.dt.float32
FP8 = mybir.dt.float8e4
NPBF16 = ml_dtypes.bfloat16
NPFP8 = ml_dtypes.float8_e4m3

DVE_CHUNKS = (4, 9)       # p-chunks evicted via VectorE stt from PSUM

_CACHE = {}


def _build():
    nc = bacc.Bacc(
        "TRN2", target_bir_lowering=False, debug=False, num_devices=NCORES
    )
    # partition-major layouts: [128, KC, cols]
    xT_d = nc.dram_tensor("xT", [128, KC, N], FP8, kind="ExternalInput")
    x2T_d = nc.dram_tensor("x2T", [128, KC, N], FP8, kind="ExternalInput")
    wT_d = nc.dram_tensor("wT", [128, KC, P_PAD], FP8, kind="ExternalInput")
    w2c_d = nc.dram_tensor("w2c", [128, PC], F32, kind="ExternalInput")
    out_d = nc.dram_tensor("out", [P, BL, HW], BF16, kind="ExternalOutput")

    IDENT = mybir.ActivationFunctionType.Identity
    COPY = mybir.ActivationFunctionType.Copy
    DR = mybir.MatmulPerfMode.DoubleRow
    ADD = mybir.AluOpType.add

    with tile.TileContext(nc) as tc:
        with (
            tc.tile_pool(name="resident", bufs=1) as rpool,
            tc.tile_pool(name="u", bufs=3) as u_pool,
            tc.tile_pool(name="z", bufs=2) as z_pool,
            tc.tile_pool(name="o", bufs=3) as o_pool,
            tc.tile_pool(name="pm", bufs=2, space=bass.MemorySpace.PSUM) as pm_pool,
        ):
            # ---- resident tiles ----
            x_sb = rpool.tile([128, KC, N], FP8, tag="x")
            x2_sb = rpool.tile([128, KC, N], FP8, tag="x2")
            wt_sb = rpool.tile([128, KC, P_PAD], FP8, tag="wt")
            ones_sb = rpool.tile([128, 2, 128], FP8, tag="ones")
            w2col = rpool.tile([128, PC], F32, tag="w2col")
            i2r = rpool.tile([128, BL, HW], BF16, tag="i2r")
            wscr = rpool.tile([128, 128], BF16, tag="wscr")

            # ones via memset (no DMA dependency -> warm-up can start at once)
            nc.gpsimd.memset(ones_sb[:], 1.0)

            # ---- PE warm-up burst: drives HAM to K=8/8 before real work ----
            wps = pm_pool.tile([128, 2, 1024], F32, tag="ps", name="warm")
            for i in range(NWARM):
                nc.tensor.matmul(
                    wps[:, 0, 0:128], ones_sb[:], ones_sb[:],
                    start=(i == 0), stop=(i == NWARM - 1),
                    perf_mode=DR,
                )
            nc.scalar.activation(wscr[:], wps[:, 0, 0:128], COPY)

            # ---- input DMAs, ordered so compute can start early ----
            # x2 first half: feeds the first i2 pair. x as ONE transfer:
            # 12.5KB contiguous rows run at full HBM rate, smaller row
            # patterns measured ~30% slower.
            nc.sync.dma_start(x2_sb[:, :, 0:2 * HW], x2T_d[:, :, 0:2 * HW])
            nc.sync.dma_start(wt_sb[:, :, 0:512], wT_d[:, :, 0:512])
            nc.sync.dma_start(x_sb[:], xT_d[:])
            nc.sync.dma_start(x2_sb[:, :, 2 * HW:N], x2T_d[:, :, 2 * HW:N])
            nc.sync.dma_start(wt_sb[:, :, 512:P_PAD], wT_d[:, :, 512:P_PAD])
            nc.sync.dma_start(w2col[:], w2c_d[:])

            def i2_pair(t):
                """i2 rows for images (2t, 2t+1) via ones.T @ x2 DoubleRow."""
                pi = pm_pool.tile([128, 2, 1024], F32, tag="ps", name="pi")
                for kk in range(2):
                    for j in range(2):
                        base = (2 * t + j) * HW
                        for off, nn in ((0, 512), (512, 272)):
                            nc.tensor.matmul(
                                pi[:, j, off:off + nn],
                                ones_sb[:],
                                x2_sb[:, 2 * kk:2 * kk + 2,
                                      base + off:base + off + nn],
                                start=(kk == 0), stop=(kk == 1),
                                perf_mode=DR,
                            )
                nc.scalar.activation(
                    i2r[:, 2 * t:2 * t + 2, :], pi[:, :, 0:HW], COPY
                )

            def main_chunk(p_i):
                M = min(128, P - p_i * 128)
                psl = slice(p_i * 128, p_i * 128 + M)
                ps = [pm_pool.tile([128, 2, 1024], F32, tag="ps", name=f"ps{t}")
                      for t in range(2)]
                z = z_pool.tile([128, BL, HW], BF16)
                u = (u_pool.tile([128, BL, HW], BF16, name="u")
                     if p_i not in DVE_CHUNKS else None)
                # image-major so each image's PSUM completes after 4 matmuls
                # and its eviction overlaps the remaining matmuls tightly
                for t in range(2):
                    for j in range(2):
                        base = (2 * t + j) * HW
                        for off, nn in ((0, 512), (512, 272)):
                            for kk in range(2):
                                nc.tensor.matmul(
                                    ps[t][:M, j, off:off + nn],
                                    wt_sb[:, 2 * kk:2 * kk + 2, psl],
                                    x_sb[:, 2 * kk:2 * kk + 2,
                                         base + off:base + off + nn],
                                    start=(kk == 0), stop=(kk == 1),
                                    perf_mode=DR,
                                )
                        if p_i in DVE_CHUNKS:
                            # VectorE stt straight from PSUM: (ps+w2)+i2r
                            img = 2 * t + j
                            nc.vector.scalar_tensor_tensor(
                                z[:M, img, :], ps[t][:M, j, 0:HW],
                                w2col[:M, p_i:p_i + 1], i2r[:M, img, :],
                                op0=ADD, op1=ADD,
                            )
                    if p_i not in DVE_CHUNKS:
                        # ScalarE pair eviction (+w2 bias); VectorE adds i2
                        nc.scalar.activation(
                            u[:M, 2 * t:2 * t + 2, :], ps[t][:M, :, 0:HW],
                            IDENT, bias=w2col[:M, p_i:p_i + 1], scale=1.0,
                        )
                def finish():
                    """z-add (a-chunks) + relu + store. Emitted one chunk
                    late so PSUM-critical DVE work stays at the queue head."""
                    if p_i not in DVE_CHUNKS and p_i != PC - 1:
                        nc.vector.tensor_add(z[:M], u[:M], i2r[:M])
                    o = o_pool.tile([128, BL, HW], BF16, name="o")
                    if p_i == PC - 1:
                        # short tail: per-image i2-add + relu + store
                        for img in range(BL):
                            nc.vector.tensor_add(
                                z[:M, img, :], u[:M, img, :], i2r[:M, img, :]
                            )
                            nc.vector.tensor_scalar_max(
                                o[:M, img, :], z[:M, img, :], 0.0
                            )
                            nc.sync.dma_start(
                                out_d[psl, img:img + 1, :],
                                o[:M, img:img + 1, :]
                            )
                    else:
                        nc.vector.tensor_scalar_max(o[:M], z[:M], 0.0)
                        nc.sync.dma_start(out_d[psl], o[:M])

                return finish

            # ---- schedule: i2 pair 0 first (x2 cols 0:HW are the first
            # DMA; images 2-3 need the late x2 upper half, so i2 pair 1 runs
            # after chunk 1's matmuls). Each chunk's z/relu/store chain is
            # emitted after the NEXT chunk's matmuls+evictions, so the
            # PSUM-freeing work is never queued behind bulk DVE work.
            i2_pair(0)
            fins = [main_chunk(0), main_chunk(1)]
            i2_pair(1)
            fins.pop(0)()                      # finish(0)
            for p_i in range(2, PC):
                fins.append(main_chunk(p_i))
                if p_i < PC - 1:
                    fins.pop(0)()              # finish(p_i - 1)
            fins.pop(1)()                      # finish(15): short tail first
            fins.pop(0)()                      # finish(14)

    nc.compile()
    return nc


def _get_nc():
    if "nc" not in _CACHE:
        _CACHE["nc"] = _build()
    return _CACHE["nc"]


def _make_in_maps(input, weights):
    x = np.asarray(input, dtype=np.float32)
    w = np.asarray(weights, dtype=np.float32).reshape(P, C)

    wm2 = (-2.0 * w).astype(NPFP8)                      # [P, C] fp8 of -2w
    wT = np.zeros((C, P_PAD), NPFP8)
    wT[:, :P] = wm2.T
    # [C, P_PAD] -> [KC, 128, P_PAD] -> partition-major [128, KC, P_PAD]
    wT = np.ascontiguousarray(wT.reshape(KC, 128, P_PAD).transpose(1, 0, 2))

    w2 = np.einsum("pc,pc->p", w.astype(np.float64), w.astype(np.float64))
    w2c = np.zeros(P_PAD, np.float32)
    w2c[:P] = w2.astype(np.float32)
    w2c = np.ascontiguousarray(w2c.reshape(PC, 128).T)  # [128, PC]

    in_maps = []
    for c in range(NCORES):
        sh = x[c * BL:(c + 1) * BL]                     # [4, 512, 28, 28]
        xt32 = np.ascontiguousarray(
            sh.transpose(1, 0, 2, 3).reshape(C, N)
        )
        xT = np.ascontiguousarray(
            xt32.astype(NPFP8).reshape(KC, 128, N).transpose(1, 0, 2)
        )
        x2T = np.ascontiguousarray(
            (xt32 * xt32).astype(NPFP8).reshape(KC, 128, N).transpose(1, 0, 2)
        )
        in_maps.append({"xT": xT, "x2T": x2T, "wT": wT, "w2c": w2c})
    return in_maps


def run(input, weights, trace=False):
    """Returns (output [32,2000,28,28] f32, BassKernelResults)."""
    nc = _get_nc()
    in_maps = _make_in_maps(input, weights)
    res = bass_utils.run_bass_kernel_spmd(
        nc, in_maps, core_ids=list(range(NCORES)), trace=trace
    )
    outs = [res.results[c]["out"] for c in range(NCORES)]   # [2000, 4, 784] bf16
    out = (
        np.stack(outs, axis=0)                              # [8, 2000, 4, 784]
        .transpose(0, 2, 1, 3)                              # [8, 4, 2000, 784]
        .astype(np.float32)
        .reshape(B, P, H, W)
    )
    return out, res


def kernel(input, weights):
    out, _ = run(input, weights, trace=False)
    return out
